# revision 1
# baseline (speedup 1.0000x reference)
"""BioBERT entity-aware enhancement kernel for 8 TRN2 NeuronCores.

Math reformulation (ent_tokens is batch-independent):
    M[s, e] = #{t : ent_tokens[e, t] == s}            (fixed [S, E] count matrix)
    entity_emb[b] = A[b]^T @ W                         A[b] = [onehot(types)|conf|1]  [7, E]
                                                       W    = [type_table; conf_w; conf_b]  [7, H]
    enhanced[b]   = hidden[b] + M @ entity_emb[b]
    entity_out[b] = (1/T) * M^T @ enhanced[b]

Data-parallel over batch: each of the 8 cores handles B/8 = 8 batches.
Per-core traffic: 12.6 MiB in (hidden) + 12.6 MiB out (enhanced)
+ 3.1 MiB out (entity_out)  ->  memory-bound at ~360 GB/s/core.
"""

import numpy as np

import concourse.bacc as bacc
import concourse.mybir as mybir
import concourse.tile as tile
from concourse.bass_utils import run_bass_kernel_spmd

B, S, H = 64, 512, 768
E, T = 128, 4
N_CORES = 8
PB = B // N_CORES          # batches per core
P = 128                    # SBUF partitions
NS = S // P                # s-chunks of 128 rows
HC = 384                   # H split into 2 chunks (psum bank = 512 f32)
NH = H // HC
F32 = mybir.dt.float32

_cache = {}


def _build_nc():
    nc = bacc.Bacc("TRN2", target_bir_lowering=False, debug=False)

    hid_d = nc.dram_tensor("hidden", [PB, S, H], F32, kind="ExternalInput").ap()
    at_d = nc.dram_tensor("a_t", [PB, 7, E], F32, kind="ExternalInput").ap()
    w_d = nc.dram_tensor("w", [7, H], F32, kind="ExternalInput").ap()
    mt_d = nc.dram_tensor("mt", [E, S], F32, kind="ExternalInput").ap()
    mq_d = nc.dram_tensor("mq", [P, NS, E], F32, kind="ExternalInput").ap()
    enh_d = nc.dram_tensor("enhanced", [PB, S, H], F32, kind="ExternalOutput").ap()
    ent_d = nc.dram_tensor("ent_out", [PB, E, H], F32, kind="ExternalOutput").ap()

    with tile.TileContext(nc) as tc:
        with (
            tc.tile_pool(name="const", bufs=1) as cpool,
            tc.tile_pool(name="hid", bufs=3) as hidp,
            tc.tile_pool(name="out", bufs=3) as outp,
            tc.tile_pool(name="ent", bufs=2) as entp,
            tc.tile_pool(name="pooled", bufs=2) as poolp,
            tc.tile_pool(name="ps_ent", bufs=2, space="PSUM") as ps_ent,
            tc.tile_pool(name="ps_enh", bufs=4, space="PSUM") as ps_enh,
            tc.tile_pool(name="ps_pool", bufs=2, space="PSUM") as ps_pool,
        ):
            # constants, loaded once
            mt_sb = cpool.tile([E, S], F32)          # M^T        [e, s]
            nc.sync.dma_start(out=mt_sb[:], in_=mt_d[:])
            mq_sb = cpool.tile([P, NS, E], F32)      # 0.25*M     [s%128, s//128, e]
            nc.sync.dma_start(out=mq_sb[:], in_=mq_d[:])
            w_sb = cpool.tile([7, H], F32)
            nc.sync.dma_start(out=w_sb[:], in_=w_d[:])
            at_sb = cpool.tile([7, PB, E], F32)      # all batches at once
            nc.sync.dma_start(out=at_sb[:], in_=at_d.rearrange("b k e -> k b e"))

            for b in range(PB):
                hid_sb = hidp.tile([P, NS, H], F32, tag="hid")
                nc.sync.dma_start(
                    out=hid_sb[:], in_=hid_d[b].rearrange("(n p) h -> p n h", p=P)
                )

                # entity_emb = A^T W    [E, H]
                ent_sb = entp.tile([E, H], F32, tag="ent")
                for c in range(NH):
                    ps = ps_ent.tile([E, HC], F32, tag="pse")
                    nc.tensor.matmul(
                        ps[:], at_sb[:, b, :], w_sb[:, c * HC:(c + 1) * HC],
                        start=True, stop=True,
                    )
                    nc.scalar.copy(ent_sb[:, c * HC:(c + 1) * HC], ps[:])

                # enhanced = hidden + M @ entity_emb
                out_sb = outp.tile([P, NS, H], F32, tag="out")
                for n in range(NS):
                    for c in range(NH):
                        ps = ps_enh.tile([P, HC], F32, tag="psh")
                        nc.tensor.matmul(
                            ps[:],
                            mt_sb[:, n * P:(n + 1) * P],
                            ent_sb[:, c * HC:(c + 1) * HC],
                            start=True, stop=True,
                        )
                        nc.vector.tensor_add(
                            out=out_sb[:, n, c * HC:(c + 1) * HC],
                            in0=hid_sb[:, n, c * HC:(c + 1) * HC],
                            in1=ps[:],
                        )
                nc.sync.dma_start(
                    out=enh_d[b].rearrange("(n p) h -> p n h", p=P), in_=out_sb[:]
                )

                # entity_out = 0.25 * M^T @ enhanced
                pool_sb = poolp.tile([E, H], F32, tag="pooled")
                for c in range(NH):
                    ps = ps_pool.tile([E, HC], F32, tag="psp")
                    for n in range(NS):
                        nc.tensor.matmul(
                            ps[:],
                            mq_sb[:, n, :],
                            out_sb[:, n, c * HC:(c + 1) * HC],
                            start=(n == 0), stop=(n == NS - 1),
                        )
                    nc.scalar.copy(pool_sb[:, c * HC:(c + 1) * HC], ps[:])
                nc.sync.dma_start(out=ent_d[b], in_=pool_sb[:])

    nc.compile()
    return nc


def _get_nc():
    if "nc" not in _cache:
        _cache["nc"] = _build_nc()
    return _cache["nc"]


def _prepare_in_maps(hidden_states, entity_types, entity_confidences, ent_tokens,
                     type_table, conf_w, conf_b):
    hidden = np.ascontiguousarray(np.asarray(hidden_states, np.float32))
    types = np.asarray(entity_types, np.int32)
    conf = np.asarray(entity_confidences, np.float32)
    toks = np.asarray(ent_tokens, np.int32)
    ttab = np.asarray(type_table, np.float32)
    cw = np.asarray(conf_w, np.float32)
    cb = np.asarray(conf_b, np.float32)

    m = np.zeros((S, E), np.float32)
    np.add.at(m, (toks.reshape(-1), np.repeat(np.arange(E), T)), 1.0)
    mt = np.ascontiguousarray(m.T)                                   # [E, S]
    mq = np.ascontiguousarray(0.25 * m.reshape(NS, P, E).transpose(1, 0, 2))

    w = np.ascontiguousarray(np.concatenate([ttab, cw.reshape(1, H), cb[None]], 0))
    a_t = np.zeros((B, 7, E), np.float32)
    a_t[np.arange(B)[:, None], types, np.arange(E)[None, :]] = 1.0
    a_t[:, 5, :] = conf
    a_t[:, 6, :] = 1.0

    hid_sh = hidden.reshape(N_CORES, PB, S, H)
    at_sh = np.ascontiguousarray(a_t.reshape(N_CORES, PB, 7, E))
    return [
        {"hidden": hid_sh[i], "a_t": at_sh[i], "w": w, "mt": mt, "mq": mq}
        for i in range(N_CORES)
    ]


def _run(in_maps, **kwargs):
    nc = _get_nc()
    return run_bass_kernel_spmd(nc, in_maps, core_ids=list(range(N_CORES)), **kwargs)


def _assemble(results):
    enhanced = np.concatenate(
        [results[i]["enhanced"] for i in range(N_CORES)], 0
    ).reshape(B, S, H)
    ent_out = np.concatenate(
        [results[i]["ent_out"] for i in range(N_CORES)], 0
    ).reshape(B, E, H)
    return enhanced, ent_out


def kernel(**inputs):
    in_maps = _prepare_in_maps(**inputs)
    res = _run(in_maps)
    return _assemble(res.results)


def kernel_profiled(**inputs):
    """Same as kernel() but with NTFF tracing; returns (outputs, BassKernelResults)."""
    in_maps = _prepare_in_maps(**inputs)
    res = _run(in_maps, trace=True)
    return _assemble(res.results), res


# revision 2
# speedup vs baseline: 1.1816x; 1.1816x over previous
"""BioBERT entity-aware enhancement kernel for 8 TRN2 NeuronCores.

Math reformulation (ent_tokens is batch-independent):
    M[s, e] = #{t : ent_tokens[e, t] == s}            (fixed [S, E] count matrix)
    entity_emb[b] = A[b]^T @ W                         A[b] = [onehot(types)|conf|1]  [7, E]
                                                       W    = [type_table; conf_w; conf_b]  [7, H]
    enhanced[b]   = hidden[b] + M @ entity_emb[b]
    entity_out[b] = (1/T) * M^T @ enhanced[b]

Data-parallel over batch: each of the 8 cores handles B/8 = 8 batches.
Per-core traffic: 12.6 MiB in (hidden) + 12.6 MiB out (enhanced)
+ 3.1 MiB out (entity_out)  ->  memory-bound at ~360 GB/s/core.

TensorEngine operands are bf16 (fp32 matmul runs at ~1/4 rate via hi/lo
passes); accumulation is f32 in PSUM and the dominant f32 hidden term is
added on the VectorEngine in f32, so `enhanced` stays near-exact.
"""

import ml_dtypes
import numpy as np

import concourse.bacc as bacc
import concourse.mybir as mybir
import concourse.tile as tile
from concourse.bass_utils import run_bass_kernel_spmd

B, S, H = 64, 512, 768
E, T = 128, 4
N_CORES = 8
PB = B // N_CORES          # batches per core
P = 128                    # SBUF partitions
NS = S // P                # s-chunks of 128 rows
HC = 384                   # H split into 2 chunks (psum bank = 512 f32)
NH = H // HC
F32 = mybir.dt.float32
BF16 = mybir.dt.bfloat16
NP_BF16 = ml_dtypes.bfloat16

_cache = {}


def _build_nc():
    nc = bacc.Bacc("TRN2", target_bir_lowering=False, debug=False)

    hid_d = nc.dram_tensor("hidden", [PB, S, H], F32, kind="ExternalInput").ap()
    at_d = nc.dram_tensor("a_t", [PB, 7, E], BF16, kind="ExternalInput").ap()
    w_d = nc.dram_tensor("w", [7, H], BF16, kind="ExternalInput").ap()
    mt_d = nc.dram_tensor("mt", [E, S], BF16, kind="ExternalInput").ap()
    mq_d = nc.dram_tensor("mq", [P, NS, E], BF16, kind="ExternalInput").ap()
    enh_d = nc.dram_tensor("enhanced", [PB, S, H], F32, kind="ExternalOutput").ap()
    ent_d = nc.dram_tensor("ent_out", [PB, E, H], F32, kind="ExternalOutput").ap()

    with tile.TileContext(nc) as tc:
        with (
            tc.tile_pool(name="const", bufs=1) as cpool,
            tc.tile_pool(name="hid", bufs=3) as hidp,
            tc.tile_pool(name="out", bufs=3) as outp,
            tc.tile_pool(name="outbf", bufs=2) as outbfp,
            tc.tile_pool(name="ent", bufs=2) as entp,
            tc.tile_pool(name="pooled", bufs=2) as poolp,
            tc.tile_pool(name="ps_ent", bufs=2, space="PSUM") as ps_ent,
            tc.tile_pool(name="ps_enh", bufs=4, space="PSUM") as ps_enh,
            tc.tile_pool(name="ps_pool", bufs=2, space="PSUM") as ps_pool,
        ):
            # constants, loaded once
            mt_sb = cpool.tile([E, S], BF16)         # M^T        [e, s]
            nc.sync.dma_start(out=mt_sb[:], in_=mt_d[:])
            mq_sb = cpool.tile([P, NS, E], BF16)     # 0.25*M     [s%128, s//128, e]
            nc.sync.dma_start(out=mq_sb[:], in_=mq_d[:])
            w_sb = cpool.tile([7, H], BF16)
            nc.sync.dma_start(out=w_sb[:], in_=w_d[:])
            at_sb = cpool.tile([7, PB, E], BF16)     # all batches at once
            nc.sync.dma_start(out=at_sb[:], in_=at_d.rearrange("b k e -> k b e"))

            for b in range(PB):
                hid_sb = hidp.tile([P, NS, H], F32, tag="hid")
                nc.sync.dma_start(
                    out=hid_sb[:], in_=hid_d[b].rearrange("(n p) h -> p n h", p=P)
                )

                # entity_emb = A^T W    [E, H]
                ent_sb = entp.tile([E, H], BF16, tag="ent")
                for c in range(NH):
                    ps = ps_ent.tile([E, HC], F32, tag="pse")
                    nc.tensor.matmul(
                        ps[:], at_sb[:, b, :], w_sb[:, c * HC:(c + 1) * HC],
                        start=True, stop=True,
                    )
                    nc.scalar.copy(ent_sb[:, c * HC:(c + 1) * HC], ps[:])

                # enhanced = hidden + M @ entity_emb
                out_sb = outp.tile([P, NS, H], F32, tag="out")
                for n in range(NS):
                    for c in range(NH):
                        ps = ps_enh.tile([P, HC], F32, tag="psh")
                        nc.tensor.matmul(
                            ps[:],
                            mt_sb[:, n * P:(n + 1) * P],
                            ent_sb[:, c * HC:(c + 1) * HC],
                            start=True, stop=True,
                        )
                        nc.vector.tensor_add(
                            out=out_sb[:, n, c * HC:(c + 1) * HC],
                            in0=hid_sb[:, n, c * HC:(c + 1) * HC],
                            in1=ps[:],
                        )
                nc.sync.dma_start(
                    out=enh_d[b].rearrange("(n p) h -> p n h", p=P), in_=out_sb[:]
                )

                # bf16 shadow of enhanced for the pooling matmul
                out_bf = outbfp.tile([P, NS, H], BF16, tag="outbf")
                nc.scalar.copy(out_bf[:], out_sb[:])

                # entity_out = 0.25 * M^T @ enhanced
                pool_sb = poolp.tile([E, H], F32, tag="pooled")
                for c in range(NH):
                    ps = ps_pool.tile([E, HC], F32, tag="psp")
                    for n in range(NS):
                        nc.tensor.matmul(
                            ps[:],
                            mq_sb[:, n, :],
                            out_bf[:, n, c * HC:(c + 1) * HC],
                            start=(n == 0), stop=(n == NS - 1),
                        )
                    nc.scalar.copy(pool_sb[:, c * HC:(c + 1) * HC], ps[:])
                nc.sync.dma_start(out=ent_d[b], in_=pool_sb[:])

    nc.compile()
    return nc


def _get_nc():
    if "nc" not in _cache:
        _cache["nc"] = _build_nc()
    return _cache["nc"]


def _prepare_in_maps(hidden_states, entity_types, entity_confidences, ent_tokens,
                     type_table, conf_w, conf_b):
    hidden = np.ascontiguousarray(np.asarray(hidden_states, np.float32))
    types = np.asarray(entity_types, np.int32)
    conf = np.asarray(entity_confidences, np.float32)
    toks = np.asarray(ent_tokens, np.int32)
    ttab = np.asarray(type_table, np.float32)
    cw = np.asarray(conf_w, np.float32)
    cb = np.asarray(conf_b, np.float32)

    m = np.zeros((S, E), np.float32)
    np.add.at(m, (toks.reshape(-1), np.repeat(np.arange(E), T)), 1.0)
    mt = np.ascontiguousarray(m.T).astype(NP_BF16)                   # [E, S]
    mq = np.ascontiguousarray(
        0.25 * m.reshape(NS, P, E).transpose(1, 0, 2)
    ).astype(NP_BF16)

    w = np.concatenate([ttab, cw.reshape(1, H), cb[None]], 0).astype(NP_BF16)
    a_t = np.zeros((B, 7, E), np.float32)
    a_t[np.arange(B)[:, None], types, np.arange(E)[None, :]] = 1.0
    a_t[:, 5, :] = conf
    a_t[:, 6, :] = 1.0
    a_t = a_t.astype(NP_BF16)

    hid_sh = hidden.reshape(N_CORES, PB, S, H)
    at_sh = np.ascontiguousarray(a_t.reshape(N_CORES, PB, 7, E))
    return [
        {"hidden": hid_sh[i], "a_t": at_sh[i], "w": w, "mt": mt, "mq": mq}
        for i in range(N_CORES)
    ]


def _run(in_maps, **kwargs):
    nc = _get_nc()
    return run_bass_kernel_spmd(nc, in_maps, core_ids=list(range(N_CORES)), **kwargs)


def _assemble(results):
    enhanced = np.concatenate(
        [results[i]["enhanced"] for i in range(N_CORES)], 0
    ).reshape(B, S, H)
    ent_out = np.concatenate(
        [results[i]["ent_out"] for i in range(N_CORES)], 0
    ).reshape(B, E, H)
    return enhanced, ent_out


def kernel(**inputs):
    in_maps = _prepare_in_maps(**inputs)
    res = _run(in_maps)
    return _assemble(res.results)


def kernel_profiled(**inputs):
    """Same as kernel() but with NTFF tracing; returns (outputs, BassKernelResults)."""
    in_maps = _prepare_in_maps(**inputs)
    res = _run(in_maps, trace=True)
    return _assemble(res.results), res


# revision 3
# speedup vs baseline: 1.2645x; 1.0702x over previous
"""BioBERT entity-aware enhancement kernel for 8 TRN2 NeuronCores.

Math reformulation (ent_tokens is batch-independent):
    M[s, e] = #{t : ent_tokens[e, t] == s}            (fixed [S, E] count matrix)
    entity_emb[b] = A[b]^T @ W                         A[b] = [onehot(types)|conf|1]  [7, E]
                                                       W    = [type_table; conf_w; conf_b]  [7, H]
    enhanced[b]   = hidden[b] + M @ entity_emb[b]
    entity_out[b] = (1/T) * M^T @ enhanced[b]

Data-parallel over batch: each of the 8 cores handles B/8 = 8 batches.
Per-core traffic: 12.6 MiB in (hidden) + 12.6 MiB out (enhanced)
+ 3.1 MiB out (entity_out)  ->  memory-bound at ~360 GB/s/core.

TensorEngine operands are bf16 (fp32 matmul runs at ~1/4 rate via hi/lo
passes); accumulation is f32 in PSUM and the dominant f32 hidden term is
added on the VectorEngine in f32, so `enhanced` stays near-exact.
"""

import ml_dtypes
import numpy as np

import concourse.bacc as bacc
import concourse.mybir as mybir
import concourse.tile as tile
from concourse.bass_utils import run_bass_kernel_spmd

B, S, H = 64, 512, 768
E, T = 128, 4
N_CORES = 8
PB = B // N_CORES          # batches per core
P = 128                    # SBUF partitions
NS = S // P                # s-chunks of 128 rows
HC = 384                   # H split into 2 chunks (psum bank = 512 f32)
NH = H // HC
F32 = mybir.dt.float32
BF16 = mybir.dt.bfloat16
NP_BF16 = ml_dtypes.bfloat16

_cache = {}


def _build_nc():
    nc = bacc.Bacc("TRN2", target_bir_lowering=False, debug=False)

    hid_d = nc.dram_tensor("hidden", [PB, S, H], F32, kind="ExternalInput").ap()
    at_d = nc.dram_tensor("a_t", [PB, 7, E], BF16, kind="ExternalInput").ap()
    w_d = nc.dram_tensor("w", [7, H], BF16, kind="ExternalInput").ap()
    mt_d = nc.dram_tensor("mt", [E, S], BF16, kind="ExternalInput").ap()
    mq_d = nc.dram_tensor("mq", [P, NS, E], BF16, kind="ExternalInput").ap()
    enh_d = nc.dram_tensor("enhanced", [PB, S, H], F32, kind="ExternalOutput").ap()
    ent_d = nc.dram_tensor("ent_out", [PB, E, H], F32, kind="ExternalOutput").ap()

    GB = 2                      # batches per DMA granule
    NG = PB // GB
    with tile.TileContext(nc) as tc:
        with (
            tc.tile_pool(name="const", bufs=1) as cpool,
            tc.tile_pool(name="hid", bufs=3) as hidp,
            tc.tile_pool(name="out", bufs=2) as outp,
            tc.tile_pool(name="outbf", bufs=2) as outbfp,
            tc.tile_pool(name="ent", bufs=2) as entp,
            tc.tile_pool(name="pooled", bufs=2) as poolp,
            tc.tile_pool(name="ps_ent", bufs=2, space="PSUM") as ps_ent,
            tc.tile_pool(name="ps_enh", bufs=4, space="PSUM") as ps_enh,
            tc.tile_pool(name="ps_pool", bufs=2, space="PSUM") as ps_pool,
        ):
            # constants, loaded once (SWDGE ring so the big hidden loads
            # on the sync HWDGE ring start immediately)
            mt_sb = cpool.tile([E, S], BF16)         # M^T        [e, s]
            nc.gpsimd.dma_start(out=mt_sb[:], in_=mt_d[:])
            mq_sb = cpool.tile([P, NS, E], BF16)     # 0.25*M     [s%128, s//128, e]
            nc.gpsimd.dma_start(out=mq_sb[:], in_=mq_d[:])
            w_sb = cpool.tile([7, H], BF16)
            nc.gpsimd.dma_start(out=w_sb[:], in_=w_d[:])
            at_sb = cpool.tile([7, PB, E], BF16)     # all batches at once
            nc.gpsimd.dma_start(out=at_sb[:], in_=at_d.rearrange("b k e -> k b e"))

            for g in range(NG):
                hid_sb = hidp.tile([P, GB, NS, H], F32, tag="hid")
                nc.sync.dma_start(
                    out=hid_sb[:],
                    in_=hid_d[g * GB:(g + 1) * GB].rearrange(
                        "b (n p) h -> p b n h", p=P
                    ),
                )

                out_sb = outp.tile([P, GB, NS, H], F32, tag="out")
                out_bf = outbfp.tile([P, GB, NS, H], BF16, tag="outbf")
                ent_sb = entp.tile([E, GB, H], BF16, tag="ent")
                pool_sb = poolp.tile([E, GB, H], F32, tag="pooled")

                for j in range(GB):
                    b = g * GB + j

                    # entity_emb = A^T W    [E, H]
                    for c in range(NH):
                        ps = ps_ent.tile([E, HC], F32, tag="pse")
                        nc.tensor.matmul(
                            ps[:], at_sb[:, b, :], w_sb[:, c * HC:(c + 1) * HC],
                            start=True, stop=True,
                        )
                        nc.scalar.copy(ent_sb[:, j, c * HC:(c + 1) * HC], ps[:])

                    # enhanced = hidden + M @ entity_emb
                    for n in range(NS):
                        for c in range(NH):
                            ps = ps_enh.tile([P, HC], F32, tag="psh")
                            nc.tensor.matmul(
                                ps[:],
                                mt_sb[:, n * P:(n + 1) * P],
                                ent_sb[:, j, c * HC:(c + 1) * HC],
                                start=True, stop=True,
                            )
                            nc.vector.tensor_add(
                                out=out_sb[:, j, n, c * HC:(c + 1) * HC],
                                in0=hid_sb[:, j, n, c * HC:(c + 1) * HC],
                                in1=ps[:],
                            )

                    # bf16 shadow of enhanced for the pooling matmul
                    nc.scalar.copy(out_bf[:, j], out_sb[:, j])

                    # entity_out = 0.25 * M^T @ enhanced
                    for c in range(NH):
                        ps = ps_pool.tile([E, HC], F32, tag="psp")
                        for n in range(NS):
                            nc.tensor.matmul(
                                ps[:],
                                mq_sb[:, n, :],
                                out_bf[:, j, n, c * HC:(c + 1) * HC],
                                start=(n == 0), stop=(n == NS - 1),
                            )
                        nc.scalar.copy(pool_sb[:, j, c * HC:(c + 1) * HC], ps[:])

                # stores on the SWDGE ring (gpsimd) to interleave with loads
                nc.gpsimd.dma_start(
                    out=enh_d[g * GB:(g + 1) * GB].rearrange(
                        "b (n p) h -> p b n h", p=P
                    ),
                    in_=out_sb[:],
                )
                nc.gpsimd.dma_start(
                    out=ent_d[g * GB:(g + 1) * GB].rearrange("b e h -> e b h"),
                    in_=pool_sb[:],
                )

    nc.compile()
    return nc


def _get_nc():
    if "nc" not in _cache:
        _cache["nc"] = _build_nc()
    return _cache["nc"]


def _prepare_in_maps(hidden_states, entity_types, entity_confidences, ent_tokens,
                     type_table, conf_w, conf_b):
    hidden = np.ascontiguousarray(np.asarray(hidden_states, np.float32))
    types = np.asarray(entity_types, np.int32)
    conf = np.asarray(entity_confidences, np.float32)
    toks = np.asarray(ent_tokens, np.int32)
    ttab = np.asarray(type_table, np.float32)
    cw = np.asarray(conf_w, np.float32)
    cb = np.asarray(conf_b, np.float32)

    m = np.zeros((S, E), np.float32)
    np.add.at(m, (toks.reshape(-1), np.repeat(np.arange(E), T)), 1.0)
    mt = np.ascontiguousarray(m.T).astype(NP_BF16)                   # [E, S]
    mq = np.ascontiguousarray(
        0.25 * m.reshape(NS, P, E).transpose(1, 0, 2)
    ).astype(NP_BF16)

    w = np.concatenate([ttab, cw.reshape(1, H), cb[None]], 0).astype(NP_BF16)
    a_t = np.zeros((B, 7, E), np.float32)
    a_t[np.arange(B)[:, None], types, np.arange(E)[None, :]] = 1.0
    a_t[:, 5, :] = conf
    a_t[:, 6, :] = 1.0
    a_t = a_t.astype(NP_BF16)

    hid_sh = hidden.reshape(N_CORES, PB, S, H)
    at_sh = np.ascontiguousarray(a_t.reshape(N_CORES, PB, 7, E))
    return [
        {"hidden": hid_sh[i], "a_t": at_sh[i], "w": w, "mt": mt, "mq": mq}
        for i in range(N_CORES)
    ]


def _run(in_maps, **kwargs):
    nc = _get_nc()
    return run_bass_kernel_spmd(nc, in_maps, core_ids=list(range(N_CORES)), **kwargs)


def _assemble(results):
    enhanced = np.concatenate(
        [results[i]["enhanced"] for i in range(N_CORES)], 0
    ).reshape(B, S, H)
    ent_out = np.concatenate(
        [results[i]["ent_out"] for i in range(N_CORES)], 0
    ).reshape(B, E, H)
    return enhanced, ent_out


def kernel(**inputs):
    in_maps = _prepare_in_maps(**inputs)
    res = _run(in_maps)
    return _assemble(res.results)


def kernel_profiled(**inputs):
    """Same as kernel() but with NTFF tracing; returns (outputs, BassKernelResults)."""
    in_maps = _prepare_in_maps(**inputs)
    res = _run(in_maps, trace=True)
    return _assemble(res.results), res
